# revision 40
# baseline (speedup 1.0000x reference)
"""ConvLSTM3D encoder for 8 trn2 NeuronCores — collective-free version.

Sharding: core c handles batch c//4, z-quarter k = c%4 (output planes
[8k, 8k+8)).  Instead of a per-step halo exchange, each core computes a
shrinking window of planes: step t computes h_t/c_t on 26-2t planes so
that after 10 steps exactly its 8 output planes are valid (halo
replication; the time loop then needs NO cross-core communication).

Sliding slot coords: at step t, slot s holds plane z = s + t + 8k - 9.
Writes of h_t[slot s] land at slots s-g for shift-group g in {0,1,2}, so
the window start stays at slot 0 every step.  Planes with z outside
[0,32) are neutralized by a host-set "poison" im2col row (-50 into all
gates -> sigmoid==0 -> h=c==0), reproducing zero padding with no
branches.

Per 4-plane slice: 9 matmul passes (3x3 y,x-deltas; 3 z-shifted h copies
on partitions 0..95 contract z in-pass; x-im2col taps + bias + poison on
partitions 96..124 ride pass 0) -> PSUM -> sigmoid/tanh into `gates`
(gate-major) -> 4 partition-crossing DMAs into `gt` (plane-major) ->
LSTM pointwise on DVE -> 3 shifted copies per plane rebuild the h stack
in place.
"""

import os
import sys
from contextlib import ExitStack

import numpy as np

for _p in ("/opt/trn_rl_repo", "/root/.axon_site/_ro/trn_rl_repo"):
    if os.path.isdir(_p) and _p not in sys.path:
        sys.path.insert(0, _p)

import concourse.bass as bass
import concourse.bacc as bacc
import concourse.mybir as mybir
from concourse import tile
from concourse.bass_utils import run_bass_kernel_spmd

F32 = mybir.dt.float32
MM_DT = mybir.dt.bfloat16  # matmul operand dtype; 2-byte => DVE 2x copies
GT_DT = mybir.dt.bfloat16  # post-activation gate dtype

T = 10
CH = 32            # hidden channels
PLW = 34           # padded plane width
PL = PLW * PLW     # padded plane elements (1156)
NSLOT = 26         # h-stack slots (t=0 window is 26 planes)
NROW = 125         # contraction rows: 96 h + 27 x-taps + ones + poison
DELTAS = [(dy, dx) for dy in range(3) for dx in range(3)]
WIDTHS = [26 - 2 * t for t in range(T)]   # computed planes per step
POISON = -50.0

_prog_cache = {}


def _slices(width):
    """(start_slot, n_planes) list for one step; 4-plane slices plus an
    optional trailing 2-plane slice (widths are always even)."""
    out = []
    s = 0
    while s < width:
        n = 4 if width - s >= 4 else width - s
        out.append((s, n))
        s += n
    return out


def _build_program():
    if "nc" in _prog_cache:
        return _prog_cache["nc"]

    nc = bacc.Bacc(num_devices=8)

    xim_d = nc.dram_tensor("xim", [T, 29, NSLOT, PL], MM_DT, kind="ExternalInput")
    whl_d = nc.dram_tensor("whl", [9, NROW, 128], MM_DT, kind="ExternalInput")
    zeros_d = nc.dram_tensor("zeros", [96, NSLOT * PL], MM_DT,
                             kind="ExternalInput")
    c0_d = nc.dram_tensor("c0", [128, 7 * 1024], F32, kind="ExternalInput")
    hout_d = nc.dram_tensor("hout", [CH, 8, 32, 32], MM_DT,
                            kind="ExternalOutput")

    # global slice schedule (for xim prefetch lookahead); step 0 is
    # precomputed on the host (depends only on x), device starts at t=1
    sched = []
    for t in range(1, T):
        if t == T - 1:
            for s0 in range(0, WIDTHS[t], 2):
                sched.append((t, s0, 2))
        else:
            for (s0, np_) in _slices(WIDTHS[t]):
                sched.append((t, s0, np_))

    with ExitStack() as ctx:
        tc = ctx.enter_context(tile.TileContext(nc))
        pers = ctx.enter_context(tc.tile_pool(name="pers", bufs=1))
        psum = ctx.enter_context(tc.tile_pool(name="psum", bufs=4, space="PSUM"))
        work = ctx.enter_context(tc.tile_pool(name="work", bufs=2))

        hstack = pers.tile([128, NSLOT * PL], MM_DT, tag="hstack", name="hstack")
        wh_sb = pers.tile([128, 9 * 128], MM_DT, tag="wh", name="wh_sb")
        cst = pers.tile([128, 7 * 1024], F32, tag="cst", name="cst")

        hsv = hstack[:, :].rearrange("p (s y x) -> p s y x", s=NSLOT, y=PLW, x=PLW)
        hsv2 = hstack[:, :].rearrange("p (s f) -> p s f", s=NSLOT, f=PL)

        for d in range(9):
            nc.sync.dma_start(out=wh_sb[0:NROW, 128 * d:128 * (d + 1)],
                              in_=whl_d[d])

        # load the host-precomputed h_0 shift-stack (borders zeroed — the
        # matmul passes read the full 34x34 padded planes) and c_0
        eighth = NSLOT * PL // 8
        for qq in range(8):
            lo = qq * eighth
            hi = NSLOT * PL if qq == 7 else (qq + 1) * eighth
            nc.sync.dma_start(out=hstack[0:96, lo:hi],
                              in_=zeros_d[:, lo:hi])
        for qq in range(4):
            nc.sync.dma_start(out=cst[:, 1792 * qq:1792 * (qq + 1)],
                              in_=c0_d[:, 1792 * qq:1792 * (qq + 1)])

        # prefetch xim for the first few slices
        for g in range(2):
            t_, s0_, np2 = sched[g]
            nc.gpsimd.dma_start(out=hsv2[96:125, s0_:s0_ + np2, :],
                                in_=xim_d[t_, :, s0_:s0_ + np2, :])

        copy_engines = [nc.vector, nc.vector, nc.vector]
        dma_engines = [nc.sync, nc.scalar, nc.gpsimd]
        eng_i = 0
        dma_i = 0

        for gi, (t, s0, npl) in enumerate(sched):
            j = s0 // 4
            PP = 32 * npl          # active partitions in plane-major layout
            nh = (npl + 1) // 2    # psum tiles in this slice
            gates = work.tile([128, 4096], GT_DT, tag="gates", name="gates", bufs=3)
            gt = work.tile([128, 4096], GT_DT, tag="gt", name="gt", bufs=4)

            # prefetch xim two slices ahead (same-slot WAR is safe: that
            # region's pass-0 matmuls are two slices old)
            if gi + 2 < len(sched):
                t_, s0_, np2 = sched[gi + 2]
                nc.gpsimd.dma_start(out=hsv2[96:125, s0_:s0_ + np2, :],
                                    in_=xim_d[t_, :, s0_:s0_ + np2, :])

            for h in range(npl):  # one psum tile per plane
                sl = s0 + h
                ps = psum.tile([128, 1024], F32, tag="ps", name="ps")
                for di, (dy, dx) in enumerate(DELTAS):
                    for cq in range(2):
                        r0 = 16 * cq
                        nc.tensor.matmul(
                            ps[:, 512 * cq:512 * (cq + 1)],
                            lhsT=wh_sb[0:NROW, 128 * di:128 * (di + 1)],
                            rhs=hsv[0:NROW, sl, r0 + dy:r0 + dy + 16,
                                    dx:dx + 32],
                            start=(di == 0), stop=(di == 8))
                span = slice(1024 * h, 1024 * (h + 1))
                nc.scalar.activation(gates[0:96, span], ps[0:96, :],
                                     mybir.ActivationFunctionType.Sigmoid)
                nc.scalar.activation(gates[96:128, span], ps[96:128, :],
                                     mybir.ActivationFunctionType.Tanh)
                # shuffle gate-major -> plane-major right behind this
                # plane's activations: gt[32q+c, G*1024+f] = gates[32G+c, q*1024+f]
                for G in range(4):
                    eng = dma_engines[dma_i % 3]
                    dma_i += 1
                    eng.dma_start(
                        out=gt[32 * h:32 * h + 32,
                               1024 * G:1024 * (G + 1)],
                        in_=gates[32 * G:32 * G + 32, span])

            i_t = gt[0:PP, 0:1024]
            f_t = gt[0:PP, 1024:2048]
            o_t = gt[0:PP, 2048:3072]
            g_t = gt[0:PP, 3072:4096]
            qb = s0 % 4            # group offset within the cst column
            c_sl = cst[32 * qb:32 * qb + PP, 1024 * j:1024 * (j + 1)]

            if t == 0:
                nc.vector.tensor_mul(c_sl, i_t, g_t)
            else:
                # c_old is shifted one slot down (sliding coords): DMA it
                # into an aligned scratch (engine APs can't start at
                # partition 32 with >32 partitions; DMA APs can)
                # needed c_old slots are 4j+1 .. 4j+npl: groups 1..npl of
                # col j (wrapping to group 0 of col j+1 only when npl == 4)
                csh = work.tile([128, 1024], F32, tag="csh", name="csh")
                if qb == 0:
                    nc.scalar.dma_start(out=csh[0:PP - 32, :],
                                        in_=cst[32:PP, 1024 * j:1024 * (j + 1)])
                    if PP == 128:
                        nc.scalar.dma_start(
                            out=csh[96:128, :],
                            in_=cst[0:32, 1024 * (j + 1):1024 * (j + 2)])
                    else:
                        nc.scalar.dma_start(
                            out=csh[PP - 32:PP, :],
                            in_=cst[PP:PP + 32, 1024 * j:1024 * (j + 1)])
                else:
                    # 2-plane slice at group offset qb: slots s0+1, s0+2
                    for g2 in (1, 2):
                        gg = qb + g2
                        nc.scalar.dma_start(
                            out=csh[32 * (g2 - 1):32 * g2, :],
                            in_=cst[32 * (gg % 4):32 * (gg % 4) + 32,
                                    1024 * (j + gg // 4):
                                    1024 * (j + gg // 4 + 1)])
                prod = work.tile([128, 1024], F32, tag="pw", name="prod")
                tmp = work.tile([128, 1024], F32, tag="pw", name="tmp")
                nc.vector.tensor_mul(prod[0:PP, :], i_t, g_t)
                nc.vector.tensor_mul(tmp[0:PP, :], f_t, csh[0:PP, :])
                nc.vector.tensor_add(c_sl, prod[0:PP, :], tmp[0:PP, :])

            tanhc = work.tile([128, 1024], F32, tag="pw", name="tanhc")
            # h_t in f32r so the h-stack copies are same-dtype and the
            # fp32r-rounding happens here (BIR requires f32r-rounded
            # producers for f32r matmul operands)
            h_t = work.tile([128, 1024], MM_DT, tag="ht", name="h_t", bufs=3)
            nc.scalar.activation(tanhc[0:PP, :], c_sl,
                                 mybir.ActivationFunctionType.Tanh)
            nc.vector.tensor_mul(h_t[0:PP, :], o_t, tanhc[0:PP, :])

            ht3 = h_t[:, :].rearrange("p (y x) -> p y x", y=32, x=32)
            if t == T - 1:
                for q in range(npl):
                    nc.sync.dma_start(out=hout_d[:, s0 + q, :, :],
                                      in_=ht3[32 * q:32 * q + 32])
            else:
                wnext = WIDTHS[t + 1]
                for q in range(npl):
                    s = s0 + q
                    for g in range(3):
                        ts = s - g
                        if 0 <= ts < wnext:
                            eng = copy_engines[eng_i % 3]
                            eng_i += 1
                            if eng is nc.scalar:
                                eng.copy(hsv[32 * g:32 * g + 32, ts, 1:33, 1:33],
                                         ht3[32 * q:32 * q + 32])
                            else:
                                eng.tensor_copy(
                                    hsv[32 * g:32 * g + 32, ts, 1:33, 1:33],
                                    ht3[32 * q:32 * q + 32])

    nc.finalize()
    _prog_cache["nc"] = nc
    return nc


def _host_inputs(input_batch, Wx, Wh, b):
    import ml_dtypes
    bf16 = ml_dtypes.bfloat16
    input_batch = np.asarray(input_batch, dtype=np.float32)
    Wx = np.asarray(Wx, dtype=np.float32)
    Wh = np.asarray(Wh, dtype=np.float32)
    b = np.asarray(b, dtype=np.float32)
    B = input_batch.shape[0]

    xp = np.zeros((B, T, 66, 66, 66), np.float32)
    xp[:, :, 1:65, 1:65, 1:65] = input_batch[:, :, 0]

    whl = np.zeros((9, NROW, 128), np.float32)
    for di, (dy, dx) in enumerate(DELTAS):
        for g in range(3):
            whl[di, 32 * g:32 * g + 32, :] = Wh[:, :, g, dy, dx].T
    whl[0, 96:123, :] = Wx[:, 0].reshape(128, 27).T
    whl[0, 123, :] = b
    whl[0, 124, :] = POISON

    in_maps = []
    for c in range(8):
        bidx, k = divmod(c, 4)
        xim = np.zeros((T, 29, NSLOT, PLW, PLW), np.float32)
        for t in range(T):
            w = WIDTHS[t]
            zbase = t + 8 * k - 9
            for s in range(w):
                z = s + zbase
                if 0 <= z < 32:
                    for tz in range(3):
                        for ty in range(3):
                            for tx in range(3):
                                tap = tz * 9 + ty * 3 + tx
                                xim[t, tap, s, 0:32, 0:32] = xp[
                                    bidx, t, 2 * z + tz,
                                    ty:ty + 64:2, tx:tx + 64:2]
                    xim[t, 27, s, 0:32, 0:32] = 1.0
                else:
                    xim[t, 28, s, 0:32, 0:32] = 1.0
        # precompute step 0 on the host (pure function of x): h_0 shipped
        # pre-packed in the 3-group shifted h-stack layout, c_0 in cst layout
        g0 = np.einsum('rsv,rp->psv',
                       xim[0, :, :, 0:32, 0:32].reshape(29, NSLOT, 1024),
                       whl[0, 96:125, :], optimize=True)
        sig = lambda x: 0.5 * (1.0 + np.tanh(0.5 * x))
        c0 = sig(g0[0:32]) * np.tanh(g0[96:128])
        h0 = sig(g0[64:96]) * np.tanh(c0)
        h0s = np.zeros((96, NSLOT, PLW, PLW), np.float32)
        for sg in range(NSLOT):
            for g in range(3):
                s = sg + g
                if s < NSLOT:
                    h0s[32 * g:32 * g + 32, sg, 1:33, 1:33] = \
                        h0[:, s].reshape(32, 32, 32)
        c0c = np.zeros((128, 7, 1024), np.float32)
        for s in range(NSLOT):
            c0c[32 * (s % 4):32 * (s % 4) + 32, s // 4] = c0[:, s]
        in_maps.append({
            "xim": xim.reshape(T, 29, NSLOT, PL).astype(bf16),
            "whl": whl.astype(bf16),
            "zeros": h0s.reshape(96, NSLOT * PL).astype(bf16),
            "c0": c0c.reshape(128, 7 * 1024),
        })
    return in_maps


def run_cores(in_maps, **kwargs):
    nc = _build_program()
    return run_bass_kernel_spmd(nc, in_maps, list(range(8)), **kwargs)


def kernel(input_batch, Wx, Wh, b):
    in_maps = _host_inputs(input_batch, Wx, Wh, b)
    res = run_cores(in_maps)
    out = np.zeros((2, CH, 32, 32, 32), np.float32)
    for c in range(8):
        bidx, k = divmod(c, 4)
        out[bidx, :, 8 * k:8 * k + 8] = np.asarray(
            res.results[c]["hout"], dtype=np.float32)
    return out


# revision 42
# speedup vs baseline: 1.0969x; 1.0969x over previous
"""ConvLSTM3D encoder for 8 trn2 NeuronCores — collective-free version.

Sharding: core c handles batch c//4, z-quarter k = c%4 (output planes
[8k, 8k+8)).  Instead of a per-step halo exchange, each core computes a
shrinking window of planes: step t computes h_t/c_t on 26-2t planes so
that after 10 steps exactly its 8 output planes are valid (halo
replication; the time loop then needs NO cross-core communication).

Sliding slot coords: at step t, slot s holds plane z = s + t + 8k - 9.
Writes of h_t[slot s] land at slots s-g for shift-group g in {0,1,2}, so
the window start stays at slot 0 every step.  Planes with z outside
[0,32) are neutralized by a host-set "poison" im2col row (-50 into all
gates -> sigmoid==0 -> h=c==0), reproducing zero padding with no
branches.

Per 4-plane slice: 9 matmul passes (3x3 y,x-deltas; 3 z-shifted h copies
on partitions 0..95 contract z in-pass; x-im2col taps + bias + poison on
partitions 96..124 ride pass 0) -> PSUM -> sigmoid/tanh into `gates`
(gate-major) -> 4 partition-crossing DMAs into `gt` (plane-major) ->
LSTM pointwise on DVE -> 3 shifted copies per plane rebuild the h stack
in place.
"""

import os
import sys
from contextlib import ExitStack

import numpy as np

for _p in ("/opt/trn_rl_repo", "/root/.axon_site/_ro/trn_rl_repo"):
    if os.path.isdir(_p) and _p not in sys.path:
        sys.path.insert(0, _p)

import concourse.bass as bass
import concourse.bacc as bacc
import concourse.mybir as mybir
from concourse import tile
from concourse.bass_utils import run_bass_kernel_spmd

F32 = mybir.dt.float32
MM_DT = mybir.dt.bfloat16  # matmul operand dtype; 2-byte => DVE 2x copies
GT_DT = mybir.dt.bfloat16  # post-activation gate dtype

T = 10
CH = 32            # hidden channels
PLW = 34           # padded plane width
PL = PLW * PLW     # padded plane elements (1156)
NSLOT = 26         # h-stack slots (t=0 window is 26 planes)
NROW = 125         # contraction rows: 96 h + 27 x-taps + ones + poison
DELTAS = [(dy, dx) for dy in range(3) for dx in range(3)]
WIDTHS = [26 - 2 * t for t in range(T)]   # computed planes per step
POISON = -50.0

_prog_cache = {}


def _slices(width):
    """(start_slot, n_planes) list for one step; 4-plane slices plus an
    optional trailing 2-plane slice (widths are always even)."""
    out = []
    s = 0
    while s < width:
        n = 4 if width - s >= 4 else width - s
        out.append((s, n))
        s += n
    return out


def _build_program():
    if "nc" in _prog_cache:
        return _prog_cache["nc"]

    nc = bacc.Bacc(num_devices=8)

    xim_d = nc.dram_tensor("xim", [T, 29, NSLOT, PL], MM_DT, kind="ExternalInput")
    whl_d = nc.dram_tensor("whl", [9, NROW, 128], MM_DT, kind="ExternalInput")
    zeros_d = nc.dram_tensor("zeros", [96, NSLOT * PL], MM_DT,
                             kind="ExternalInput")
    c0_d = nc.dram_tensor("c0", [128, 7 * 1024], F32, kind="ExternalInput")
    hout_d = nc.dram_tensor("hout", [CH, 8, 32, 32], MM_DT,
                            kind="ExternalOutput")

    # global slice schedule (for xim prefetch lookahead); step 0 is
    # precomputed on the host (depends only on x), device starts at t=1
    sched = []
    for t in range(1, T):
        for (s0, np_) in _slices(WIDTHS[t]):
            sched.append((t, s0, np_))

    with ExitStack() as ctx:
        tc = ctx.enter_context(tile.TileContext(nc))
        pers = ctx.enter_context(tc.tile_pool(name="pers", bufs=1))
        psum = ctx.enter_context(tc.tile_pool(name="psum", bufs=4, space="PSUM"))
        work = ctx.enter_context(tc.tile_pool(name="work", bufs=2))

        hstack = pers.tile([128, NSLOT * PL], MM_DT, tag="hstack", name="hstack")
        wh_sb = pers.tile([128, 9 * 128], MM_DT, tag="wh", name="wh_sb")
        cst = pers.tile([128, 7 * 1024], F32, tag="cst", name="cst")

        hsv = hstack[:, :].rearrange("p (s y x) -> p s y x", s=NSLOT, y=PLW, x=PLW)
        hsv2 = hstack[:, :].rearrange("p (s f) -> p s f", s=NSLOT, f=PL)

        for d in range(9):
            nc.sync.dma_start(out=wh_sb[0:NROW, 128 * d:128 * (d + 1)],
                              in_=whl_d[d])

        # load the host-precomputed h_0 shift-stack (borders zeroed — the
        # matmul passes read the full 34x34 padded planes) and c_0
        eighth = NSLOT * PL // 8
        for qq in range(8):
            lo = qq * eighth
            hi = NSLOT * PL if qq == 7 else (qq + 1) * eighth
            nc.sync.dma_start(out=hstack[0:96, lo:hi],
                              in_=zeros_d[:, lo:hi])
        for qq in range(4):
            nc.sync.dma_start(out=cst[:, 1792 * qq:1792 * (qq + 1)],
                              in_=c0_d[:, 1792 * qq:1792 * (qq + 1)])

        # prefetch xim for the first few slices
        for g in range(2):
            t_, s0_, np2 = sched[g]
            nc.gpsimd.dma_start(out=hsv2[96:125, s0_:s0_ + np2, :],
                                in_=xim_d[t_, :, s0_:s0_ + np2, :])

        copy_engines = [nc.vector, nc.vector, nc.vector]
        dma_engines = [nc.sync, nc.scalar, nc.gpsimd]
        eng_i = 0
        dma_i = 0

        for gi, (t, s0, npl) in enumerate(sched):
            j = s0 // 4
            PP = 32 * npl          # active partitions in plane-major layout
            nh = (npl + 1) // 2    # psum tiles in this slice
            gates = work.tile([128, 4096], GT_DT, tag="gates", name="gates")
            gt = work.tile([128, 4096], GT_DT, tag="gt", name="gt", bufs=4)

            # prefetch xim two slices ahead (same-slot WAR is safe: that
            # region's pass-0 matmuls are two slices old)
            if gi + 2 < len(sched):
                t_, s0_, np2 = sched[gi + 2]
                nc.gpsimd.dma_start(out=hsv2[96:125, s0_:s0_ + np2, :],
                                    in_=xim_d[t_, :, s0_:s0_ + np2, :])

            qb = s0 % 4
            csh = work.tile([128, 1024], F32, tag="csh", name="csh", bufs=3)
            if t > 0:
                # c_old (shifted one slot down) fetched early: its only dep
                # is the previous step's cst columns, so it overlaps the
                # matmuls instead of gating the pointwise
                nc.scalar.dma_start(out=csh[0:PP - 32, :],
                                    in_=cst[32:PP, 1024 * j:1024 * (j + 1)])
                if PP == 128:
                    nc.scalar.dma_start(
                        out=csh[96:128, :],
                        in_=cst[0:32, 1024 * (j + 1):1024 * (j + 2)])
                else:
                    nc.scalar.dma_start(
                        out=csh[PP - 32:PP, :],
                        in_=cst[PP:PP + 32, 1024 * j:1024 * (j + 1)])

            for h in range(npl):  # one psum tile per plane
                sl = s0 + h
                ps = psum.tile([128, 1024], F32, tag="ps", name="ps")
                for di, (dy, dx) in enumerate(DELTAS):
                    for cq in range(2):
                        r0 = 16 * cq
                        nc.tensor.matmul(
                            ps[:, 512 * cq:512 * (cq + 1)],
                            lhsT=wh_sb[0:NROW, 128 * di:128 * (di + 1)],
                            rhs=hsv[0:NROW, sl, r0 + dy:r0 + dy + 16,
                                    dx:dx + 32],
                            start=(di == 0), stop=(di == 8))
                span = slice(1024 * h, 1024 * (h + 1))
                nc.scalar.activation(gates[0:96, span], ps[0:96, :],
                                     mybir.ActivationFunctionType.Sigmoid)
                nc.scalar.activation(gates[96:128, span], ps[96:128, :],
                                     mybir.ActivationFunctionType.Tanh)
                # shuffle gate-major -> plane-major right behind this
                # plane's activations: gt[32q+c, G*1024+f] = gates[32G+c, q*1024+f]
                for G in range(4):
                    eng = dma_engines[dma_i % 3]
                    dma_i += 1
                    eng.dma_start(
                        out=gt[32 * h:32 * h + 32,
                               1024 * G:1024 * (G + 1)],
                        in_=gates[32 * G:32 * G + 32, span])

            i_t = gt[0:PP, 0:1024]
            f_t = gt[0:PP, 1024:2048]
            o_t = gt[0:PP, 2048:3072]
            g_t = gt[0:PP, 3072:4096]
            c_sl = cst[32 * qb:32 * qb + PP, 1024 * j:1024 * (j + 1)]

            if t == 0:
                nc.vector.tensor_mul(c_sl, i_t, g_t)
            else:
                prod = work.tile([128, 1024], F32, tag="pw", name="prod")
                tmp = work.tile([128, 1024], F32, tag="pw", name="tmp")
                nc.vector.tensor_mul(prod[0:PP, :], i_t, g_t)
                nc.vector.tensor_mul(tmp[0:PP, :], f_t, csh[0:PP, :])
                nc.vector.tensor_add(c_sl, prod[0:PP, :], tmp[0:PP, :])

            tanhc = work.tile([128, 1024], F32, tag="pw", name="tanhc")
            # h_t in f32r so the h-stack copies are same-dtype and the
            # fp32r-rounding happens here (BIR requires f32r-rounded
            # producers for f32r matmul operands)
            h_t = work.tile([128, 1024], MM_DT, tag="ht", name="h_t")
            nc.scalar.activation(tanhc[0:PP, :], c_sl,
                                 mybir.ActivationFunctionType.Tanh)
            nc.vector.tensor_mul(h_t[0:PP, :], o_t, tanhc[0:PP, :])

            ht3 = h_t[:, :].rearrange("p (y x) -> p y x", y=32, x=32)
            if t == T - 1:
                for q in range(npl):
                    nc.sync.dma_start(out=hout_d[:, s0 + q, :, :],
                                      in_=ht3[32 * q:32 * q + 32])
            else:
                wnext = WIDTHS[t + 1]
                for q in range(npl):
                    s = s0 + q
                    for g in range(3):
                        ts = s - g
                        if 0 <= ts < wnext:
                            eng = copy_engines[eng_i % 3]
                            eng_i += 1
                            if eng is nc.scalar:
                                eng.copy(hsv[32 * g:32 * g + 32, ts, 1:33, 1:33],
                                         ht3[32 * q:32 * q + 32])
                            else:
                                eng.tensor_copy(
                                    hsv[32 * g:32 * g + 32, ts, 1:33, 1:33],
                                    ht3[32 * q:32 * q + 32])

    nc.finalize()
    _prog_cache["nc"] = nc
    return nc


def _host_inputs(input_batch, Wx, Wh, b):
    import ml_dtypes
    bf16 = ml_dtypes.bfloat16
    input_batch = np.asarray(input_batch, dtype=np.float32)
    Wx = np.asarray(Wx, dtype=np.float32)
    Wh = np.asarray(Wh, dtype=np.float32)
    b = np.asarray(b, dtype=np.float32)
    B = input_batch.shape[0]

    xp = np.zeros((B, T, 66, 66, 66), np.float32)
    xp[:, :, 1:65, 1:65, 1:65] = input_batch[:, :, 0]

    whl = np.zeros((9, NROW, 128), np.float32)
    for di, (dy, dx) in enumerate(DELTAS):
        for g in range(3):
            whl[di, 32 * g:32 * g + 32, :] = Wh[:, :, g, dy, dx].T
    whl[0, 96:123, :] = Wx[:, 0].reshape(128, 27).T
    whl[0, 123, :] = b
    whl[0, 124, :] = POISON

    in_maps = []
    for c in range(8):
        bidx, k = divmod(c, 4)
        xim = np.zeros((T, 29, NSLOT, PLW, PLW), np.float32)
        for t in range(T):
            w = WIDTHS[t]
            zbase = t + 8 * k - 9
            for s in range(w):
                z = s + zbase
                if 0 <= z < 32:
                    for tz in range(3):
                        for ty in range(3):
                            for tx in range(3):
                                tap = tz * 9 + ty * 3 + tx
                                xim[t, tap, s, 0:32, 0:32] = xp[
                                    bidx, t, 2 * z + tz,
                                    ty:ty + 64:2, tx:tx + 64:2]
                    xim[t, 27, s, 0:32, 0:32] = 1.0
                else:
                    xim[t, 28, s, 0:32, 0:32] = 1.0
        # precompute step 0 on the host (pure function of x): h_0 shipped
        # pre-packed in the 3-group shifted h-stack layout, c_0 in cst layout
        g0 = np.einsum('rsv,rp->psv',
                       xim[0, :, :, 0:32, 0:32].reshape(29, NSLOT, 1024),
                       whl[0, 96:125, :], optimize=True)
        sig = lambda x: 0.5 * (1.0 + np.tanh(0.5 * x))
        c0 = sig(g0[0:32]) * np.tanh(g0[96:128])
        h0 = sig(g0[64:96]) * np.tanh(c0)
        h0s = np.zeros((96, NSLOT, PLW, PLW), np.float32)
        for sg in range(NSLOT):
            for g in range(3):
                s = sg + g
                if s < NSLOT:
                    h0s[32 * g:32 * g + 32, sg, 1:33, 1:33] = \
                        h0[:, s].reshape(32, 32, 32)
        c0c = np.zeros((128, 7, 1024), np.float32)
        for s in range(NSLOT):
            c0c[32 * (s % 4):32 * (s % 4) + 32, s // 4] = c0[:, s]
        in_maps.append({
            "xim": xim.reshape(T, 29, NSLOT, PL).astype(bf16),
            "whl": whl.astype(bf16),
            "zeros": h0s.reshape(96, NSLOT * PL).astype(bf16),
            "c0": c0c.reshape(128, 7 * 1024),
        })
    return in_maps


def run_cores(in_maps, **kwargs):
    nc = _build_program()
    return run_bass_kernel_spmd(nc, in_maps, list(range(8)), **kwargs)


def kernel(input_batch, Wx, Wh, b):
    in_maps = _host_inputs(input_batch, Wx, Wh, b)
    res = run_cores(in_maps)
    out = np.zeros((2, CH, 32, 32, 32), np.float32)
    for c in range(8):
        bidx, k = divmod(c, 4)
        out[bidx, :, 8 * k:8 * k + 8] = np.asarray(
            res.results[c]["hout"], dtype=np.float32)
    return out
